# revision 47
# baseline (speedup 1.0000x reference)
"""Mixture-of-Experts (top-2 of 8) Trainium2 kernel, expert-parallel over 8 NeuronCores.

Strategy (per the expert-parallel sharding hint):
  Launch A (data-parallel gating): each core computes gating logits for T/8
    tokens. x is host-split into exact bf16 hi/lo halves so the logit
    matmul runs at bf16 PE rate with fp32-equivalent accuracy
    (xh*wh + xl*wh + xh*wl; dropped xl*wl ~2^-18; verified zero top-k
    flips vs the fp32 reference on randn data). Top-2 selection +
    renormalized combine weights use the identity
      renorm_top2_w(e) = sigmoid(2*logit_e - m1 - m2)   for selected e,
    which equals exp(l_e)/(exp(m1)+exp(m2)) exactly for e in the top-2.
    Output: dense [T, E] combine weights (zero for unselected experts).
  Host routing ("all-to-all dispatch"): from the device-computed combine
    weights, build per-expert token index lists, gather+transpose+bf16-cast
    the routed tokens for each expert, pad to a common capacity C.
  Launch B (expert-parallel FFN): core e holds expert e's weights. Computes
    h^T = gelu(W1^T x^T + b1), y^T = (W2^T h^T + b2) * w on the PE in bf16
    with fp32 accumulation; biases added exactly in fp32 on the scalar
    engine; combine weight applied on the vector engine. Weights are
    pre-packed host-side into partition-major layouts so DMA descriptors
    are large and the first matmul starts early.
  Host unshard: scatter-add the 8 weighted partial outputs into [T, D].

All floating-point math of the reference model (gating softmax/top-k/renorm,
FFN matmuls, gelu, biases, combine weighting) is computed on device; the host
only makes routing/sharding decisions and moves data.
"""

import os
import sys
import types

import numpy as np
import ml_dtypes

import concourse.bass as bass
import concourse.mybir as mybir
import concourse.tile as tile
from concourse import bacc
from concourse.bass_utils import run_bass_kernel_spmd
from concourse.masks import make_identity

N_CORES = 8
P = 128
B, S, D, H, E = 2, 2048, 1024, 4096, 8
T = B * S
TG = T // N_CORES  # tokens per core for gating
BF16 = ml_dtypes.bfloat16

AF = mybir.ActivationFunctionType
ALU = mybir.AluOpType
AX = mybir.AxisListType
F32 = mybir.dt.float32
BF = mybir.dt.bfloat16


def _install_profile_hook():
    """Register the antenv.axon_hooks NTFF hook this image lacks, so
    BASS_TRACE=1 profiling works. Harmless no-op on failure."""
    try:
        if "antenv.axon_hooks" in sys.modules:
            return
        import antenv
        from trn_agent_boot.trn_boot import _ntff_profile_via_ctypes

        mod = types.ModuleType("antenv.axon_hooks")
        _h = [None]
        mod.set_axon_ntff_profile_hook = lambda h: _h.__setitem__(0, h)
        mod.get_axon_ntff_profile_hook = lambda: _h[0]
        sys.modules["antenv.axon_hooks"] = mod
        antenv.axon_hooks = mod
        so = "/opt/axon/libaxon_pjrt.so"
        if os.path.exists(so):
            mod.set_axon_ntff_profile_hook(_ntff_profile_via_ctypes(so))
    except Exception:
        pass


_install_profile_hook()

_NC_CACHE = {}


def _build_gate_nc():
    """Launch A: per-core gating for TG tokens.

    Inputs : xtg [D, TG] f32 (token slice, transposed; d = p*8+kd mapping)
             wgr [P, KD, E] f32 (host-packed Wg: wgr[p,kd,e] = Wg[p*8+kd, e])
    Output : wout [P, TT, E] f32 — renormalized top-2 combine weights for
             token tt*128+p, dense over E (zero where expert not selected).
    """
    key = ("gate", TG)
    if key in _NC_CACHE:
        return _NC_CACHE[key]
    nc = bacc.Bacc("TRN2", target_bir_lowering=False, debug=False, num_devices=N_CORES)
    KD = D // P
    TT = TG // P
    xhg = nc.dram_tensor("xhg", [D, TG], BF, kind="ExternalInput")
    xlg = nc.dram_tensor("xlg", [D, TG], BF, kind="ExternalInput")
    wgh = nc.dram_tensor("wgh", [P, KD, E], BF, kind="ExternalInput")
    wgl = nc.dram_tensor("wgl", [P, KD, E], BF, kind="ExternalInput")
    wout = nc.dram_tensor("wout", [P, TT, E], F32, kind="ExternalOutput")
    with tile.TileContext(nc) as tc:
        with (
            tc.tile_pool(name="cst", bufs=1) as cst,
            tc.tile_pool(name="wk", bufs=2) as wk,
            tc.tile_pool(name="ps", bufs=1, space="PSUM") as ps,
        ):
            # x is host-split into exact bf16 hi/lo halves so the gating
            # matmul runs at bf16 rate (1cy/col vs fp32's 4): logits =
            # xh*wh + xl*wh + xh*wl, dropped xl*wl term is ~2^-18.
            wgh_sb = cst.tile([P, KD, E], BF)
            nc.sync.dma_start(wgh_sb[:], wgh.ap())
            wgl_sb = cst.tile([P, KD, E], BF)
            nc.sync.dma_start(wgl_sb[:], wgl.ap())
            ident = cst.tile([E, E], F32)
            make_identity(nc, ident[:])
            xh_ap = xhg.ap().rearrange("(p kd) t -> p (kd t)", p=P)
            xl_ap = xlg.ap().rearrange("(p kd) t -> p (kd t)", p=P)
            xh_sb = cst.tile([P, KD * TG], BF)
            xl_sb = cst.tile([P, KD * TG], BF)
            for kd in range(KD):
                eng_h = nc.sync if kd % 2 == 0 else nc.scalar
                eng_l = nc.scalar if kd % 2 == 0 else nc.sync
                sl = slice(kd * TG, (kd + 1) * TG)
                eng_h.dma_start(xh_sb[:, sl], xh_ap[:, sl])
                eng_l.dma_start(xl_sb[:, sl], xl_ap[:, sl])
            # Warm-up: preload the sigmoid act table (covers copy/identity
            # too) and ramp the PE clock p-state with dummy matmuls while
            # the x DMA streams in.
            wz = cst.tile([P, 512], F32)
            nc.vector.memset(wz[:], 0.0)
            sgd = wk.tile([P, 1], F32, tag="sgd")
            nc.scalar.activation(sgd[:], wz[:, 0:1], AF.Sigmoid)
            pw = ps.tile([E, 512], F32, tag="pw")
            for i in range(9):
                nc.tensor.matmul(
                    pw[:, :P], wz[:, 0:E], wz[:, :P], start=True, stop=True
                )
            # Wg stationary (cheap LDWEIGHTS), tokens moving: one psum
            # accumulator [E, TG]; per-kd matmuls pipeline with the x DMA.
            pl = ps.tile([E, TG], F32, tag="pl")
            for kd in range(KD):
                sl = slice(kd * TG, (kd + 1) * TG)
                nc.tensor.matmul(
                    pl[:], wgh_sb[:, kd, :], xh_sb[:, sl],
                    start=(kd == 0), stop=False,
                )
                nc.tensor.matmul(
                    pl[:], wgh_sb[:, kd, :], xl_sb[:, sl],
                    start=False, stop=False,
                )
                nc.tensor.matmul(
                    pl[:], wgl_sb[:, kd, :], xh_sb[:, sl],
                    start=False, stop=(kd == KD - 1),
                )
            wn_all = cst.tile([P, TT, E], F32)
            for tt in range(TT):
                # per-tile psum->sbuf copy (on the DVE so the scalar engine
                # only runs sigmoids) + transpose so tile 0's top-k chain
                # starts before the whole logits row is drained
                lt = wk.tile([E, P], F32, tag=f"lt{tt}", name=f"lt{tt}")
                nc.vector.tensor_scalar_add(lt[:], pl[:, tt * P : (tt + 1) * P], 0.0)
                pg = ps.tile([P, E], F32, tag=f"pg{tt}", name=f"pg{tt}")
                nc.tensor.transpose(pg[:], lt[:], ident[:])
                top8 = wk.tile([P, 8], F32, tag="t8")
                nc.vector.max(out=top8[:], in_=pg[:])
                # negthr = -(m1 + m2)
                negthr = wk.tile([P, 1], F32, tag="nt")
                nc.gpsimd.tensor_scalar(
                    out=negthr[:],
                    in0=top8[:, 0:1],
                    scalar1=top8[:, 1:2],
                    scalar2=-1.0,
                    op0=ALU.add,
                    op1=ALU.mult,
                )
                # sigmoid(2*l - m1 - m2) = exp(l)/(exp(m1)+exp(m2)) on top-2
                sg = wk.tile([P, E], F32, tag="sg")
                nc.scalar.activation(
                    sg[:], pg[:], AF.Sigmoid, bias=negthr[:], scale=2.0
                )
                mask = wk.tile([P, E], F32, tag="mk")
                nc.vector.tensor_scalar(
                    out=mask[:],
                    in0=pg[:],
                    scalar1=top8[:, 1:2],
                    scalar2=None,
                    op0=ALU.is_ge,
                )
                nc.vector.tensor_mul(wn_all[:, tt, :], sg[:], mask[:])
            nc.sync.dma_start(wout.ap(), wn_all[:])
    nc.compile()
    _NC_CACHE[key] = nc
    return nc


def _build_ffn_nc(C):
    """Launch B: per-core expert FFN over C (padded) routed tokens.

    All bulk inputs are host-packed so every dma is 128 FAT descriptors
    (per-partition contiguous), sidestepping the ~100 descr/us HWDGE
    descriptor-generation limit.

    Inputs : xt  [P, KD*C] bf16 — routed tokens (xt[p, kd*C+c] = x[kd*128+p, c])
             w1h [P, KD*256] bf16 — first 256 H-cols of W1, chunk-packed
             w1r [P, 8, KD*512] bf16 — remaining W1 in 8 chunk-packed slabs
                 (last slab half garbage: H-256 = 7.5*512)
             w2r [P, 4, KH*256] bf16 — W2 in 4 chunk-packed slabs
             b1r [P, H/P] f32, b2r [P, D/P] f32 — biases, partition-major
             wc [P, C] f32 — combine weights, replicated across partitions
    Output : yt [D, C] bf16 — w * (gelu(x W1 + b1) W2 + b2), transposed
    """
    key = ("ffn", C)
    if key in _NC_CACHE:
        return _NC_CACHE[key]
    assert C % 4 == 0
    KD = D // P  # 8 k-tiles over D
    KH = H // P  # 32 k-tiles over H
    HEAD = 256
    h_chunks = [HEAD] + [512] * 7 + [256]
    assert sum(h_chunks) == H
    DC = 256  # d columns per W2 dma chunk
    n_off = list(range(0, C, 512))
    n_szs = [min(512, C - o) for o in n_off]
    NCH = len(n_off)

    nc = bacc.Bacc("TRN2", target_bir_lowering=False, debug=False, num_devices=N_CORES)
    xt = nc.dram_tensor("xt", [P, KD * C], BF, kind="ExternalInput")
    w1h = nc.dram_tensor("w1h", [P, KD * HEAD], BF, kind="ExternalInput")
    w1r = nc.dram_tensor("w1r", [P, 8, KD * 512], BF, kind="ExternalInput")
    w2r = nc.dram_tensor("w2r", [P, 4, KH * DC], BF, kind="ExternalInput")
    b1r = nc.dram_tensor("b1r", [P, H // P], F32, kind="ExternalInput")
    b2r = nc.dram_tensor("b2r", [P, D // P], F32, kind="ExternalInput")
    wc = nc.dram_tensor("wc", [P, C], F32, kind="ExternalInput")
    yt = nc.dram_tensor("yt", [D, C], BF, kind="ExternalOutput")

    with tile.TileContext(nc) as tc:
        with (
            tc.tile_pool(name="cst", bufs=1) as cst,
            tc.tile_pool(name="w1p", bufs=3) as w1p,
            tc.tile_pool(name="w2p", bufs=2) as w2p,
            tc.tile_pool(name="outp", bufs=4) as outp,
            tc.tile_pool(name="ps", bufs=4, space="PSUM") as ps,
        ):
            # Startup loads: w1 head first on sync; thin per-kd xt slices
            # split across BOTH HWDGE rings (the scalar engine is idle until
            # the first gelu ~18us in, so early DMA issues there are free —
            # only mid-kernel scalar-ring weight streams collide with gelu).
            w1_c0 = w1p.tile([P, KD * HEAD], BF, tag="w1c0", name="w1_c0")
            xt_sb = cst.tile([P, KD * C], BF)
            nc.sync.dma_start(w1_c0[:], w1h.ap())
            for kd in range(KD):
                eng = nc.sync if kd % 2 == 0 else nc.scalar
                eng.dma_start(
                    xt_sb[:, kd * C : (kd + 1) * C],
                    xt.ap()[:, kd * C : (kd + 1) * C],
                )
            b1_sb = cst.tile([P, H // P], F32)
            nc.sync.dma_start(b1_sb[:], b1r.ap())
            b2_sb = cst.tile([P, D // P], F32)
            nc.sync.dma_start(b2_sb[:], b2r.ap())
            wc_sb = cst.tile([P, C], F32)
            nc.gpsimd.dma_start(wc_sb[:], wc.ap())
            ht_sb = cst.tile([P, KH, C], BF)
            # Warm-up: preload the gelu act table (set also covers identity)
            # and ramp the PE p-state with dummy matmuls during the DMA ramp.
            wz = cst.tile([P, 512], BF)
            nc.vector.memset(wz[:], 0.0)
            gld = cst.tile([P, 1], F32)
            nc.scalar.activation(gld[:], wz[:, 0:1], AF.Gelu)
            pw = ps.tile([P, 512], F32, tag="ps1", name="pwarm")
            for i in range(14):
                nc.tensor.matmul(
                    pw[:], wz[:, 0:P], wz[:], start=True, stop=True
                )

            # ---- mm1: ht[h, c] = gelu(sum_d w1[d, h] * xt[d, c] + b1[h]) ----
            h_tile = 0
            for hc, hsz in enumerate(h_chunks):
                if hc == 0:
                    w1_c = w1_c0
                    cs = HEAD  # chunk stride between kd slices
                else:
                    w1_c = w1p.tile([P, KD * 512], BF, tag="w1c", name=f"w1_c{hc}")
                    cs = 512
                    nc.sync.dma_start(w1_c[:], w1r.ap()[:, hc - 1, :])
                for hs in range(hsz // P):
                    psum_ts = [
                        ps.tile([P, 512], F32, tag="ps1", name=f"ps1_{h_tile}_{n}")
                        for n in range(NCH)
                    ]
                    for kd in range(KD):
                        for n in range(NCH):
                            nc.tensor.matmul(
                                psum_ts[n][:, : n_szs[n]],
                                w1_c[:, kd * cs + hs * P : kd * cs + (hs + 1) * P],
                                xt_sb[
                                    :, kd * C + n_off[n] : kd * C + n_off[n] + n_szs[n]
                                ],
                                start=(kd == 0),
                                stop=(kd == KD - 1),
                            )
                    for n in range(NCH):
                        nc.scalar.activation(
                            ht_sb[:, h_tile, n_off[n] : n_off[n] + n_szs[n]],
                            psum_ts[n][:, : n_szs[n]],
                            AF.Gelu,
                            bias=b1_sb[:, h_tile : h_tile + 1],
                        )
                    h_tile += 1

            # ---- mm2: yt[d, c] = (sum_h w2[h, d] * ht[h, c] + b2[d]) * wc[c] ----
            yt_ap = yt.ap().rearrange("(dt p) c -> p dt c", p=P)
            for dc in range(D // DC):
                w2_c = w2p.tile([P, KH * DC], BF, tag="w2c")
                nc.sync.dma_start(w2_c[:], w2r.ap()[:, dc, :])
                for dsx in range(DC // P):
                    d_tile = dc * (DC // P) + dsx
                    psum_ts = [
                        ps.tile([P, 512], F32, tag="ps2", name=f"ps2_{d_tile}_{n}")
                        for n in range(NCH)
                    ]
                    for kh in range(KH):
                        for n in range(NCH):
                            nc.tensor.matmul(
                                psum_ts[n][:, : n_szs[n]],
                                w2_c[:, kh * DC + dsx * P : kh * DC + (dsx + 1) * P],
                                ht_sb[:, kh, n_off[n] : n_off[n] + n_szs[n]],
                                start=(kh == 0),
                                stop=(kh == KH - 1),
                            )
                    out_t = outp.tile([P, C], BF, tag="out")
                    for n in range(NCH):
                        nsz = n_szs[n]
                        tmp = outp.tile([P, 512], F32, tag="tmp")
                        nc.scalar.activation(
                            tmp[:, :nsz],
                            psum_ts[n][:, :nsz],
                            AF.Identity,
                            bias=b2_sb[:, d_tile : d_tile + 1],
                        )
                        nc.vector.tensor_mul(
                            out_t[:, n_off[n] : n_off[n] + nsz],
                            tmp[:, :nsz],
                            wc_sb[:, n_off[n] : n_off[n] + nsz],
                        )
                    # scalar ring: the sync ring still streams w2 here, and
                    # queueing the outputs behind it delays the final store
                    nc.scalar.dma_start(yt_ap[:, d_tile, :], out_t[:])
    nc.compile()
    _NC_CACHE[key] = nc
    return nc


# results of the most recent kernel() call, for test harness introspection
last_results = {}


def kernel(**inputs):
    x = np.asarray(inputs["x"], np.float32)
    Wg = np.asarray(inputs["Wg"], np.float32)
    W1 = np.asarray(inputs["W1"], np.float32)
    b1 = np.asarray(inputs["b1"], np.float32)
    W2 = np.asarray(inputs["W2"], np.float32)
    b2 = np.asarray(inputs["b2"], np.float32)
    assert x.shape == (B, S, D) and Wg.shape == (D, E)
    assert W1.shape == (E, D, H) and W2.shape == (E, H, D)

    KD = D // P
    KH = H // P
    TT = TG // P
    HEAD = 256
    xf = np.ascontiguousarray(x.reshape(T, D))
    core_ids = list(range(N_CORES))

    # ---- Launch A: gating on device (data-parallel over tokens) ----
    ncA = _build_gate_nc()
    wgr = np.ascontiguousarray(Wg.reshape(P, KD, E))  # wgr[p,kd,e] = Wg[p*8+kd,e]
    wgh = wgr.astype(BF16)
    wgl = (wgr - wgh.astype(np.float32)).astype(BF16)
    in_maps_a = []
    for m in range(N_CORES):
        xs = np.ascontiguousarray(xf[m * TG : (m + 1) * TG].T)
        xh = xs.astype(BF16)
        xl = (xs - xh.astype(np.float32)).astype(BF16)
        in_maps_a.append({"xhg": xh, "xlg": xl, "wgh": wgh, "wgl": wgl})
    resA = run_bass_kernel_spmd(ncA, in_maps_a, core_ids=core_ids)
    w_full = np.concatenate(
        [
            resA.results[m]["wout"].transpose(1, 0, 2).reshape(TG, E)
            for m in range(N_CORES)
        ],
        axis=0,
    )

    # ---- Host routing: build per-expert token lists from device weights ----
    idx_list, wval_list = [], []
    max_cnt = 1
    for e in range(E):
        idx = np.nonzero(w_full[:, e] > 0.0)[0]
        idx_list.append(idx)
        wval_list.append(w_full[idx, e].astype(np.float32))
        max_cnt = max(max_cnt, len(idx))
    C = ((max_cnt + 3) // 4) * 4

    # ---- Launch B: expert-parallel FFN ----
    ncB = _build_ffn_nc(C)
    in_maps_b = []
    DC = 256
    for e in range(E):
        idx = idx_list[e]
        cnt = len(idx)
        xt = np.zeros((P, KD, C), BF16)
        xt[:, :, :cnt] = (
            xf[idx].T.astype(BF16).reshape(KD, P, cnt).transpose(1, 0, 2)
        )
        wcv = np.zeros((C,), np.float32)
        wcv[:cnt] = wval_list[e]
        w1b = W1[e].astype(BF16)  # [D, H]
        w2b = W2[e].astype(BF16)  # [H, D]
        # w1 rest chunk-packed: [P, 8 slabs, KD*512], last slab half garbage
        w1t = np.concatenate(
            [w1b[:, HEAD:], np.zeros((D, 256), BF16)], axis=1
        )  # [D, 8*512]
        w1r = (
            w1t.reshape(KD, P, 8, 512).transpose(1, 2, 0, 3).reshape(P, 8, KD * 512)
        )
        w2r = (
            w2b.reshape(KH, P, D // DC, DC)
            .transpose(1, 2, 0, 3)
            .reshape(P, D // DC, KH * DC)
        )
        in_maps_b.append(
            {
                "xt": xt.reshape(P, KD * C),
                "w1h": np.ascontiguousarray(
                    w1b[:, :HEAD].reshape(KD, P, HEAD).transpose(1, 0, 2)
                ).reshape(P, KD * HEAD),
                "w1r": np.ascontiguousarray(w1r),
                "w2r": np.ascontiguousarray(w2r),
                "b1r": np.ascontiguousarray(b1[e].reshape(H // P, P).T),
                "b2r": np.ascontiguousarray(b2[e].reshape(D // P, P).T),
                "wc": np.ascontiguousarray(np.broadcast_to(wcv, (P, C))),
            }
        )
    resB = run_bass_kernel_spmd(ncB, in_maps_b, core_ids=core_ids)

    # ---- Host unshard: scatter-add weighted partial outputs ----
    out = np.zeros((T, D), np.float32)
    for e in range(E):
        idx = idx_list[e]
        cnt = len(idx)
        if cnt:
            out[idx] += resB.results[e]["yt"][:, :cnt].T.astype(np.float32)

    last_results["gate"] = resA
    last_results["ffn"] = resB
    return out.reshape(B, S, D)


# revision 48
# speedup vs baseline: 1.0065x; 1.0065x over previous
"""Mixture-of-Experts (top-2 of 8) Trainium2 kernel, expert-parallel over 8 NeuronCores.

Strategy (per the expert-parallel sharding hint):
  Launch A (data-parallel gating): each core computes gating logits for T/8
    tokens. x is host-split into exact bf16 hi/lo halves so the logit
    matmul runs at bf16 PE rate with fp32-equivalent accuracy
    (xh*wh + xl*wh + xh*wl; dropped xl*wl ~2^-18; verified zero top-k
    flips vs the fp32 reference on randn data). Top-2 selection +
    renormalized combine weights use the identity
      renorm_top2_w(e) = sigmoid(2*logit_e - m1 - m2)   for selected e,
    which equals exp(l_e)/(exp(m1)+exp(m2)) exactly for e in the top-2.
    Output: dense [T, E] combine weights (zero for unselected experts).
  Host routing ("all-to-all dispatch"): from the device-computed combine
    weights, build per-expert token index lists, gather+transpose+bf16-cast
    the routed tokens for each expert, pad to a common capacity C.
  Launch B (expert-parallel FFN): core e holds expert e's weights. Computes
    h^T = gelu(W1^T x^T + b1), y^T = (W2^T h^T + b2) * w on the PE in bf16
    with fp32 accumulation; biases added exactly in fp32 on the scalar
    engine; combine weight applied on the vector engine. Weights are
    pre-packed host-side into partition-major layouts so DMA descriptors
    are large and the first matmul starts early.
  Host unshard: scatter-add the 8 weighted partial outputs into [T, D].

All floating-point math of the reference model (gating softmax/top-k/renorm,
FFN matmuls, gelu, biases, combine weighting) is computed on device; the host
only makes routing/sharding decisions and moves data.
"""

import os
import sys
import types

import numpy as np
import ml_dtypes

import concourse.bass as bass
import concourse.mybir as mybir
import concourse.tile as tile
from concourse import bacc
from concourse.bass_utils import run_bass_kernel_spmd
from concourse.masks import make_identity

N_CORES = 8
P = 128
B, S, D, H, E = 2, 2048, 1024, 4096, 8
T = B * S
TG = T // N_CORES  # tokens per core for gating
BF16 = ml_dtypes.bfloat16

AF = mybir.ActivationFunctionType
ALU = mybir.AluOpType
AX = mybir.AxisListType
F32 = mybir.dt.float32
BF = mybir.dt.bfloat16


def _install_profile_hook():
    """Register the antenv.axon_hooks NTFF hook this image lacks, so
    BASS_TRACE=1 profiling works. Harmless no-op on failure."""
    try:
        if "antenv.axon_hooks" in sys.modules:
            return
        import antenv
        from trn_agent_boot.trn_boot import _ntff_profile_via_ctypes

        mod = types.ModuleType("antenv.axon_hooks")
        _h = [None]
        mod.set_axon_ntff_profile_hook = lambda h: _h.__setitem__(0, h)
        mod.get_axon_ntff_profile_hook = lambda: _h[0]
        sys.modules["antenv.axon_hooks"] = mod
        antenv.axon_hooks = mod
        so = "/opt/axon/libaxon_pjrt.so"
        if os.path.exists(so):
            mod.set_axon_ntff_profile_hook(_ntff_profile_via_ctypes(so))
    except Exception:
        pass


_install_profile_hook()

_NC_CACHE = {}


def _build_gate_nc():
    """Launch A: per-core gating for TG tokens.

    Inputs : xtg [D, TG] f32 (token slice, transposed; d = p*8+kd mapping)
             wgr [P, KD, E] f32 (host-packed Wg: wgr[p,kd,e] = Wg[p*8+kd, e])
    Output : wout [P, TT, E] f32 — renormalized top-2 combine weights for
             token tt*128+p, dense over E (zero where expert not selected).
    """
    key = ("gate", TG)
    if key in _NC_CACHE:
        return _NC_CACHE[key]
    nc = bacc.Bacc("TRN2", target_bir_lowering=False, debug=False, num_devices=N_CORES)
    KD = D // P
    TT = TG // P
    xhg = nc.dram_tensor("xhg", [D, TG], BF, kind="ExternalInput")
    xlg = nc.dram_tensor("xlg", [D, TG], BF, kind="ExternalInput")
    wgh = nc.dram_tensor("wgh", [P, KD, E], BF, kind="ExternalInput")
    wgl = nc.dram_tensor("wgl", [P, KD, E], BF, kind="ExternalInput")
    wout = nc.dram_tensor("wout", [P, TT, E], F32, kind="ExternalOutput")
    with tile.TileContext(nc) as tc:
        with (
            tc.tile_pool(name="cst", bufs=1) as cst,
            tc.tile_pool(name="wk", bufs=2) as wk,
            tc.tile_pool(name="ps", bufs=1, space="PSUM") as ps,
        ):
            # x is host-split into exact bf16 hi/lo halves so the gating
            # matmul runs at bf16 rate (1cy/col vs fp32's 4): logits =
            # xh*wh + xl*wh + xh*wl, dropped xl*wl term is ~2^-18.
            wgh_sb = cst.tile([P, KD, E], BF)
            nc.sync.dma_start(wgh_sb[:], wgh.ap())
            wgl_sb = cst.tile([P, KD, E], BF)
            nc.sync.dma_start(wgl_sb[:], wgl.ap())
            ident = cst.tile([E, E], F32)
            make_identity(nc, ident[:])
            xh_ap = xhg.ap().rearrange("(p kd) t -> p (kd t)", p=P)
            xl_ap = xlg.ap().rearrange("(p kd) t -> p (kd t)", p=P)
            xh_sb = cst.tile([P, KD * TG], BF)
            xl_sb = cst.tile([P, KD * TG], BF)
            for kd in range(KD):
                eng_h = nc.sync if kd % 2 == 0 else nc.scalar
                eng_l = nc.scalar if kd % 2 == 0 else nc.sync
                sl = slice(kd * TG, (kd + 1) * TG)
                eng_h.dma_start(xh_sb[:, sl], xh_ap[:, sl])
                eng_l.dma_start(xl_sb[:, sl], xl_ap[:, sl])
            # Warm-up: preload the sigmoid act table (covers copy/identity
            # too) and ramp the PE clock p-state with dummy matmuls while
            # the x DMA streams in.
            wz = cst.tile([P, 512], F32)
            nc.vector.memset(wz[:], 0.0)
            sgd = wk.tile([P, 1], F32, tag="sgd")
            nc.scalar.activation(sgd[:], wz[:, 0:1], AF.Sigmoid)
            pw = ps.tile([E, 512], F32, tag="pw")
            for i in range(9):
                nc.tensor.matmul(
                    pw[:, :P], wz[:, 0:E], wz[:, :P], start=True, stop=True
                )
            # Wg stationary (cheap LDWEIGHTS), tokens moving: one psum
            # accumulator [E, TG]; per-kd matmuls pipeline with the x DMA.
            pl = ps.tile([E, TG], F32, tag="pl")
            for kd in range(KD):
                sl = slice(kd * TG, (kd + 1) * TG)
                nc.tensor.matmul(
                    pl[:], wgh_sb[:, kd, :], xh_sb[:, sl],
                    start=(kd == 0), stop=False,
                )
                nc.tensor.matmul(
                    pl[:], wgh_sb[:, kd, :], xl_sb[:, sl],
                    start=False, stop=False,
                )
                nc.tensor.matmul(
                    pl[:], wgl_sb[:, kd, :], xh_sb[:, sl],
                    start=False, stop=(kd == KD - 1),
                )
            wn_all = cst.tile([P, TT, E], F32)
            for tt in range(TT):
                # per-tile psum->sbuf copy (on the DVE so the scalar engine
                # only runs sigmoids) + transpose so tile 0's top-k chain
                # starts before the whole logits row is drained
                lt = wk.tile([E, P], F32, tag=f"lt{tt}", name=f"lt{tt}")
                nc.vector.tensor_scalar_add(lt[:], pl[:, tt * P : (tt + 1) * P], 0.0)
                pg = ps.tile([P, E], F32, tag=f"pg{tt}", name=f"pg{tt}")
                nc.tensor.transpose(pg[:], lt[:], ident[:])
                top8 = wk.tile([P, 8], F32, tag="t8")
                nc.vector.max(out=top8[:], in_=pg[:])
                # negthr = -(m1 + m2)
                negthr = wk.tile([P, 1], F32, tag="nt")
                nc.gpsimd.tensor_scalar(
                    out=negthr[:],
                    in0=top8[:, 0:1],
                    scalar1=top8[:, 1:2],
                    scalar2=-1.0,
                    op0=ALU.add,
                    op1=ALU.mult,
                )
                # sigmoid(2*l - m1 - m2) = exp(l)/(exp(m1)+exp(m2)) on top-2
                sg = wk.tile([P, E], F32, tag="sg")
                nc.scalar.activation(
                    sg[:], pg[:], AF.Sigmoid, bias=negthr[:], scale=2.0
                )
                mask = wk.tile([P, E], F32, tag="mk")
                nc.vector.tensor_scalar(
                    out=mask[:],
                    in0=pg[:],
                    scalar1=top8[:, 1:2],
                    scalar2=None,
                    op0=ALU.is_ge,
                )
                nc.vector.tensor_mul(wn_all[:, tt, :], sg[:], mask[:])
            nc.sync.dma_start(wout.ap(), wn_all[:])
    nc.compile()
    _NC_CACHE[key] = nc
    return nc


def _build_ffn_nc(C):
    """Launch B: per-core expert FFN over C (padded) routed tokens.

    All bulk inputs are host-packed so every dma is 128 FAT descriptors
    (per-partition contiguous), sidestepping the ~100 descr/us HWDGE
    descriptor-generation limit.

    Inputs : xt  [P, KD*C] bf16 — routed tokens (xt[p, kd*C+c] = x[kd*128+p, c])
             w1h [P, KD*256] bf16 — first 256 H-cols of W1, chunk-packed
             w1r [P, 8, KD*512] bf16 — remaining W1 in 8 chunk-packed slabs
                 (last slab half garbage: H-256 = 7.5*512)
             w2r [P, 4, KH*256] bf16 — W2 in 4 chunk-packed slabs
             b1r [P, H/P] f32, b2r [P, D/P] f32 — biases, partition-major
             wc [P, C] f32 — combine weights, replicated across partitions
    Output : yt [D, C] bf16 — w * (gelu(x W1 + b1) W2 + b2), transposed
    """
    key = ("ffn", C)
    if key in _NC_CACHE:
        return _NC_CACHE[key]
    assert C % 4 == 0
    KD = D // P  # 8 k-tiles over D
    KH = H // P  # 32 k-tiles over H
    HEAD = 256
    h_chunks = [HEAD] + [512] * 7 + [256]
    assert sum(h_chunks) == H
    DC = 256  # d columns per W2 dma chunk
    n_off = list(range(0, C, 512))
    n_szs = [min(512, C - o) for o in n_off]
    NCH = len(n_off)

    nc = bacc.Bacc("TRN2", target_bir_lowering=False, debug=False, num_devices=N_CORES)
    xt = nc.dram_tensor("xt", [P, KD * C], BF, kind="ExternalInput")
    w1h = nc.dram_tensor("w1h", [P, KD * HEAD], BF, kind="ExternalInput")
    w1r = nc.dram_tensor("w1r", [P, 8, KD * 512], BF, kind="ExternalInput")
    w2r = nc.dram_tensor("w2r", [P, 4, KH * DC], BF, kind="ExternalInput")
    b1r = nc.dram_tensor("b1r", [P, H // P], F32, kind="ExternalInput")
    b2r = nc.dram_tensor("b2r", [P, D // P], F32, kind="ExternalInput")
    wc = nc.dram_tensor("wc", [P, C], F32, kind="ExternalInput")
    yt = nc.dram_tensor("yt", [D, C], BF, kind="ExternalOutput")

    with tile.TileContext(nc) as tc:
        with (
            tc.tile_pool(name="cst", bufs=1) as cst,
            tc.tile_pool(name="w1p", bufs=3) as w1p,
            tc.tile_pool(name="w2p", bufs=2) as w2p,
            tc.tile_pool(name="outp", bufs=4) as outp,
            tc.tile_pool(name="ps", bufs=4, space="PSUM") as ps,
        ):
            # Startup loads: w1 head first, then thin per-kd xt slices, all
            # on the sync ring — measured faster than splitting across
            # rings (the scalar ring starts later and streams slower).
            w1_c0 = w1p.tile([P, KD * HEAD], BF, tag="w1c0", name="w1_c0")
            xt_sb = cst.tile([P, KD * C], BF)
            nc.sync.dma_start(w1_c0[:], w1h.ap())
            for kd in range(KD):
                nc.sync.dma_start(
                    xt_sb[:, kd * C : (kd + 1) * C],
                    xt.ap()[:, kd * C : (kd + 1) * C],
                )
            b1_sb = cst.tile([P, H // P], F32)
            nc.sync.dma_start(b1_sb[:], b1r.ap())
            b2_sb = cst.tile([P, D // P], F32)
            nc.sync.dma_start(b2_sb[:], b2r.ap())
            wc_sb = cst.tile([P, C], F32)
            nc.gpsimd.dma_start(wc_sb[:], wc.ap())
            ht_sb = cst.tile([P, KH, C], BF)
            # Warm-up: preload the gelu act table (set also covers identity)
            # and ramp the PE p-state with dummy matmuls during the DMA ramp.
            wz = cst.tile([P, 512], BF)
            nc.vector.memset(wz[:], 0.0)
            gld = cst.tile([P, 1], F32)
            nc.scalar.activation(gld[:], wz[:, 0:1], AF.Gelu)
            pw = ps.tile([P, 512], F32, tag="ps1", name="pwarm")
            for i in range(14):
                nc.tensor.matmul(
                    pw[:], wz[:, 0:P], wz[:], start=True, stop=True
                )

            # ---- mm1: ht[h, c] = gelu(sum_d w1[d, h] * xt[d, c] + b1[h]) ----
            h_tile = 0
            for hc, hsz in enumerate(h_chunks):
                if hc == 0:
                    w1_c = w1_c0
                    cs = HEAD  # chunk stride between kd slices
                else:
                    w1_c = w1p.tile([P, KD * 512], BF, tag="w1c", name=f"w1_c{hc}")
                    cs = 512
                    nc.sync.dma_start(w1_c[:], w1r.ap()[:, hc - 1, :])
                for hs in range(hsz // P):
                    psum_ts = [
                        ps.tile([P, 512], F32, tag="ps1", name=f"ps1_{h_tile}_{n}")
                        for n in range(NCH)
                    ]
                    for kd in range(KD):
                        for n in range(NCH):
                            nc.tensor.matmul(
                                psum_ts[n][:, : n_szs[n]],
                                w1_c[:, kd * cs + hs * P : kd * cs + (hs + 1) * P],
                                xt_sb[
                                    :, kd * C + n_off[n] : kd * C + n_off[n] + n_szs[n]
                                ],
                                start=(kd == 0),
                                stop=(kd == KD - 1),
                            )
                    for n in range(NCH):
                        nc.scalar.activation(
                            ht_sb[:, h_tile, n_off[n] : n_off[n] + n_szs[n]],
                            psum_ts[n][:, : n_szs[n]],
                            AF.Gelu,
                            bias=b1_sb[:, h_tile : h_tile + 1],
                        )
                    h_tile += 1

            # ---- mm2: yt[d, c] = (sum_h w2[h, d] * ht[h, c] + b2[d]) * wc[c] ----
            yt_ap = yt.ap().rearrange("(dt p) c -> p dt c", p=P)
            for dc in range(D // DC):
                w2_c = w2p.tile([P, KH * DC], BF, tag="w2c")
                nc.sync.dma_start(w2_c[:], w2r.ap()[:, dc, :])
                for dsx in range(DC // P):
                    d_tile = dc * (DC // P) + dsx
                    psum_ts = [
                        ps.tile([P, 512], F32, tag="ps2", name=f"ps2_{d_tile}_{n}")
                        for n in range(NCH)
                    ]
                    for kh in range(KH):
                        for n in range(NCH):
                            nc.tensor.matmul(
                                psum_ts[n][:, : n_szs[n]],
                                w2_c[:, kh * DC + dsx * P : kh * DC + (dsx + 1) * P],
                                ht_sb[:, kh, n_off[n] : n_off[n] + n_szs[n]],
                                start=(kh == 0),
                                stop=(kh == KH - 1),
                            )
                    out_t = outp.tile([P, C], BF, tag="out")
                    for n in range(NCH):
                        nsz = n_szs[n]
                        tmp = outp.tile([P, 512], F32, tag="tmp")
                        nc.scalar.activation(
                            tmp[:, :nsz],
                            psum_ts[n][:, :nsz],
                            AF.Identity,
                            bias=b2_sb[:, d_tile : d_tile + 1],
                        )
                        nc.vector.tensor_mul(
                            out_t[:, n_off[n] : n_off[n] + nsz],
                            tmp[:, :nsz],
                            wc_sb[:, n_off[n] : n_off[n] + nsz],
                        )
                    # scalar ring: the sync ring still streams w2 here, and
                    # queueing the outputs behind it delays the final store
                    nc.scalar.dma_start(yt_ap[:, d_tile, :], out_t[:])
    nc.compile()
    _NC_CACHE[key] = nc
    return nc


# results of the most recent kernel() call, for test harness introspection
last_results = {}


def kernel(**inputs):
    x = np.asarray(inputs["x"], np.float32)
    Wg = np.asarray(inputs["Wg"], np.float32)
    W1 = np.asarray(inputs["W1"], np.float32)
    b1 = np.asarray(inputs["b1"], np.float32)
    W2 = np.asarray(inputs["W2"], np.float32)
    b2 = np.asarray(inputs["b2"], np.float32)
    assert x.shape == (B, S, D) and Wg.shape == (D, E)
    assert W1.shape == (E, D, H) and W2.shape == (E, H, D)

    KD = D // P
    KH = H // P
    TT = TG // P
    HEAD = 256
    xf = np.ascontiguousarray(x.reshape(T, D))
    core_ids = list(range(N_CORES))

    # ---- Launch A: gating on device (data-parallel over tokens) ----
    ncA = _build_gate_nc()
    wgr = np.ascontiguousarray(Wg.reshape(P, KD, E))  # wgr[p,kd,e] = Wg[p*8+kd,e]
    wgh = wgr.astype(BF16)
    wgl = (wgr - wgh.astype(np.float32)).astype(BF16)
    in_maps_a = []
    for m in range(N_CORES):
        xs = np.ascontiguousarray(xf[m * TG : (m + 1) * TG].T)
        xh = xs.astype(BF16)
        xl = (xs - xh.astype(np.float32)).astype(BF16)
        in_maps_a.append({"xhg": xh, "xlg": xl, "wgh": wgh, "wgl": wgl})
    resA = run_bass_kernel_spmd(ncA, in_maps_a, core_ids=core_ids)
    w_full = np.concatenate(
        [
            resA.results[m]["wout"].transpose(1, 0, 2).reshape(TG, E)
            for m in range(N_CORES)
        ],
        axis=0,
    )

    # ---- Host routing: build per-expert token lists from device weights ----
    idx_list, wval_list = [], []
    max_cnt = 1
    for e in range(E):
        idx = np.nonzero(w_full[:, e] > 0.0)[0]
        idx_list.append(idx)
        wval_list.append(w_full[idx, e].astype(np.float32))
        max_cnt = max(max_cnt, len(idx))
    C = ((max_cnt + 3) // 4) * 4

    # ---- Launch B: expert-parallel FFN ----
    ncB = _build_ffn_nc(C)
    in_maps_b = []
    DC = 256
    for e in range(E):
        idx = idx_list[e]
        cnt = len(idx)
        xt = np.zeros((P, KD, C), BF16)
        xt[:, :, :cnt] = (
            xf[idx].T.astype(BF16).reshape(KD, P, cnt).transpose(1, 0, 2)
        )
        wcv = np.zeros((C,), np.float32)
        wcv[:cnt] = wval_list[e]
        w1b = W1[e].astype(BF16)  # [D, H]
        w2b = W2[e].astype(BF16)  # [H, D]
        # w1 rest chunk-packed: [P, 8 slabs, KD*512], last slab half garbage
        w1t = np.concatenate(
            [w1b[:, HEAD:], np.zeros((D, 256), BF16)], axis=1
        )  # [D, 8*512]
        w1r = (
            w1t.reshape(KD, P, 8, 512).transpose(1, 2, 0, 3).reshape(P, 8, KD * 512)
        )
        w2r = (
            w2b.reshape(KH, P, D // DC, DC)
            .transpose(1, 2, 0, 3)
            .reshape(P, D // DC, KH * DC)
        )
        in_maps_b.append(
            {
                "xt": xt.reshape(P, KD * C),
                "w1h": np.ascontiguousarray(
                    w1b[:, :HEAD].reshape(KD, P, HEAD).transpose(1, 0, 2)
                ).reshape(P, KD * HEAD),
                "w1r": np.ascontiguousarray(w1r),
                "w2r": np.ascontiguousarray(w2r),
                "b1r": np.ascontiguousarray(b1[e].reshape(H // P, P).T),
                "b2r": np.ascontiguousarray(b2[e].reshape(D // P, P).T),
                "wc": np.ascontiguousarray(np.broadcast_to(wcv, (P, C))),
            }
        )
    resB = run_bass_kernel_spmd(ncB, in_maps_b, core_ids=core_ids)

    # ---- Host unshard: scatter-add weighted partial outputs ----
    out = np.zeros((T, D), np.float32)
    for e in range(E):
        idx = idx_list[e]
        cnt = len(idx)
        if cnt:
            out[idx] += resB.results[e]["yt"][:, :cnt].T.astype(np.float32)

    last_results["gate"] = resA
    last_results["ffn"] = resB
    return out.reshape(B, S, D)


# revision 51
# speedup vs baseline: 1.0073x; 1.0008x over previous
"""Mixture-of-Experts (top-2 of 8) Trainium2 kernel, expert-parallel over 8 NeuronCores.

Strategy (per the expert-parallel sharding hint):
  Launch A (data-parallel gating): each core computes gating logits for T/8
    tokens. x is host-split into exact bf16 hi/lo halves so the logit
    matmul runs at bf16 PE rate with fp32-equivalent accuracy
    (xh*wh + xl*wh + xh*wl; dropped xl*wl ~2^-18; verified zero top-k
    flips vs the fp32 reference on randn data). Top-2 selection +
    renormalized combine weights use the identity
      renorm_top2_w(e) = sigmoid(2*logit_e - m1 - m2)   for selected e,
    which equals exp(l_e)/(exp(m1)+exp(m2)) exactly for e in the top-2.
    Output: dense [T, E] combine weights (zero for unselected experts).
  Host routing ("all-to-all dispatch"): from the device-computed combine
    weights, build per-expert token index lists, gather+transpose+bf16-cast
    the routed tokens for each expert, pad to a common capacity C.
  Launch B (expert-parallel FFN): core e holds expert e's weights. Computes
    h^T = gelu(W1^T x^T + b1), y^T = (W2^T h^T + b2) * w on the PE in bf16
    with fp32 accumulation; biases added exactly in fp32 on the scalar
    engine; combine weight applied on the vector engine. Weights are
    pre-packed host-side into partition-major layouts so DMA descriptors
    are large and the first matmul starts early.
  Host unshard: scatter-add the 8 weighted partial outputs into [T, D].

All floating-point math of the reference model (gating softmax/top-k/renorm,
FFN matmuls, gelu, biases, combine weighting) is computed on device; the host
only makes routing/sharding decisions and moves data.
"""

import os
import sys
import types

import numpy as np
import ml_dtypes

import concourse.bass as bass
import concourse.mybir as mybir
import concourse.tile as tile
from concourse import bacc
from concourse.bass_utils import run_bass_kernel_spmd
from concourse.masks import make_identity

N_CORES = 8
P = 128
B, S, D, H, E = 2, 2048, 1024, 4096, 8
T = B * S
TG = T // N_CORES  # tokens per core for gating
BF16 = ml_dtypes.bfloat16

AF = mybir.ActivationFunctionType
ALU = mybir.AluOpType
AX = mybir.AxisListType
F32 = mybir.dt.float32
BF = mybir.dt.bfloat16


def _install_profile_hook():
    """Register the antenv.axon_hooks NTFF hook this image lacks, so
    BASS_TRACE=1 profiling works. Harmless no-op on failure."""
    try:
        if "antenv.axon_hooks" in sys.modules:
            return
        import antenv
        from trn_agent_boot.trn_boot import _ntff_profile_via_ctypes

        mod = types.ModuleType("antenv.axon_hooks")
        _h = [None]
        mod.set_axon_ntff_profile_hook = lambda h: _h.__setitem__(0, h)
        mod.get_axon_ntff_profile_hook = lambda: _h[0]
        sys.modules["antenv.axon_hooks"] = mod
        antenv.axon_hooks = mod
        so = "/opt/axon/libaxon_pjrt.so"
        if os.path.exists(so):
            mod.set_axon_ntff_profile_hook(_ntff_profile_via_ctypes(so))
    except Exception:
        pass


_install_profile_hook()

_NC_CACHE = {}


def _build_gate_nc():
    """Launch A: per-core gating for TG tokens.

    Inputs : xtg [D, TG] f32 (token slice, transposed; d = p*8+kd mapping)
             wgr [P, KD, E] f32 (host-packed Wg: wgr[p,kd,e] = Wg[p*8+kd, e])
    Output : wout [P, TT, E] f32 — renormalized top-2 combine weights for
             token tt*128+p, dense over E (zero where expert not selected).
    """
    key = ("gate", TG)
    if key in _NC_CACHE:
        return _NC_CACHE[key]
    nc = bacc.Bacc("TRN2", target_bir_lowering=False, debug=False, num_devices=N_CORES)
    KD = D // P
    TT = TG // P
    xhg = nc.dram_tensor("xhg", [D, TG], BF, kind="ExternalInput")
    xlg = nc.dram_tensor("xlg", [D, TG], BF, kind="ExternalInput")
    wgh = nc.dram_tensor("wgh", [P, KD, E], BF, kind="ExternalInput")
    wgl = nc.dram_tensor("wgl", [P, KD, E], BF, kind="ExternalInput")
    wout = nc.dram_tensor("wout", [P, TT, E], F32, kind="ExternalOutput")
    with tile.TileContext(nc) as tc:
        with (
            tc.tile_pool(name="cst", bufs=1) as cst,
            tc.tile_pool(name="wk", bufs=2) as wk,
            tc.tile_pool(name="ps", bufs=1, space="PSUM") as ps,
        ):
            # x is host-split into exact bf16 hi/lo halves so the gating
            # matmul runs at bf16 rate (1cy/col vs fp32's 4): logits =
            # xh*wh + xl*wh + xh*wl, dropped xl*wl term is ~2^-18.
            wgh_sb = cst.tile([P, KD, E], BF)
            nc.sync.dma_start(wgh_sb[:], wgh.ap())
            wgl_sb = cst.tile([P, KD, E], BF)
            nc.sync.dma_start(wgl_sb[:], wgl.ap())
            ident = cst.tile([E, E], F32)
            make_identity(nc, ident[:])
            xh_ap = xhg.ap().rearrange("(p kd) t -> p (kd t)", p=P)
            xl_ap = xlg.ap().rearrange("(p kd) t -> p (kd t)", p=P)
            xh_sb = cst.tile([P, KD * TG], BF)
            xl_sb = cst.tile([P, KD * TG], BF)
            for kd in range(KD):
                eng_h = nc.sync if kd % 2 == 0 else nc.scalar
                eng_l = nc.scalar if kd % 2 == 0 else nc.sync
                sl = slice(kd * TG, (kd + 1) * TG)
                eng_h.dma_start(xh_sb[:, sl], xh_ap[:, sl])
                eng_l.dma_start(xl_sb[:, sl], xl_ap[:, sl])
            # Warm-up: preload the sigmoid act table (covers copy/identity
            # too) and ramp the PE clock p-state with dummy matmuls while
            # the x DMA streams in.
            wz = cst.tile([P, 512], F32)
            nc.vector.memset(wz[:], 0.0)
            sgd = wk.tile([P, 1], F32, tag="sgd")
            nc.scalar.activation(sgd[:], wz[:, 0:1], AF.Sigmoid)
            pw = ps.tile([E, 512], F32, tag="pw")
            for i in range(9):
                nc.tensor.matmul(
                    pw[:, :P], wz[:, 0:E], wz[:, :P], start=True, stop=True
                )
            # Wg stationary (cheap LDWEIGHTS), tokens moving: one psum
            # accumulator [E, TG]; per-kd matmuls pipeline with the x DMA.
            pl = ps.tile([E, TG], F32, tag="pl")
            for kd in range(KD):
                sl = slice(kd * TG, (kd + 1) * TG)
                nc.tensor.matmul(
                    pl[:], wgh_sb[:, kd, :], xh_sb[:, sl],
                    start=(kd == 0), stop=False,
                )
                nc.tensor.matmul(
                    pl[:], wgh_sb[:, kd, :], xl_sb[:, sl],
                    start=False, stop=False,
                )
                nc.tensor.matmul(
                    pl[:], wgl_sb[:, kd, :], xh_sb[:, sl],
                    start=False, stop=(kd == KD - 1),
                )
            wn_all = cst.tile([P, TT, E], F32)
            for tt in range(TT):
                # per-tile psum->sbuf copy (on the DVE so the scalar engine
                # only runs sigmoids) + transpose so tile 0's top-k chain
                # starts before the whole logits row is drained
                lt = wk.tile([E, P], F32, tag=f"lt{tt}", name=f"lt{tt}")
                if tt % 2 == 0:
                    nc.vector.tensor_scalar_add(
                        lt[:], pl[:, tt * P : (tt + 1) * P], 0.0
                    )
                else:
                    # scalar engine is idle until the first sigmoid; its act
                    # table (sigmoid set) already contains copy
                    nc.scalar.copy(lt[:], pl[:, tt * P : (tt + 1) * P])
                pg = ps.tile([P, E], F32, tag=f"pg{tt}", name=f"pg{tt}")
                nc.tensor.transpose(pg[:], lt[:], ident[:])
                top8 = wk.tile([P, 8], F32, tag="t8")
                nc.vector.max(out=top8[:], in_=pg[:])
                # negthr = -(m1 + m2)
                negthr = wk.tile([P, 1], F32, tag="nt")
                nc.gpsimd.tensor_scalar(
                    out=negthr[:],
                    in0=top8[:, 0:1],
                    scalar1=top8[:, 1:2],
                    scalar2=-1.0,
                    op0=ALU.add,
                    op1=ALU.mult,
                )
                # sigmoid(2*l - m1 - m2) = exp(l)/(exp(m1)+exp(m2)) on top-2
                sg = wk.tile([P, E], F32, tag="sg")
                nc.scalar.activation(
                    sg[:], pg[:], AF.Sigmoid, bias=negthr[:], scale=2.0
                )
                mask = wk.tile([P, E], F32, tag="mk")
                nc.vector.tensor_scalar(
                    out=mask[:],
                    in0=pg[:],
                    scalar1=top8[:, 1:2],
                    scalar2=None,
                    op0=ALU.is_ge,
                )
                nc.vector.tensor_mul(wn_all[:, tt, :], sg[:], mask[:])
                # store each half as soon as its tiles are combined
                if tt == 1:
                    nc.sync.dma_start(wout.ap()[:, 0:2, :], wn_all[:, 0:2, :])
                elif tt == TT - 1:
                    nc.sync.dma_start(wout.ap()[:, 2:, :], wn_all[:, 2:, :])
    nc.compile()
    _NC_CACHE[key] = nc
    return nc


def _build_ffn_nc(C):
    """Launch B: per-core expert FFN over C (padded) routed tokens.

    All bulk inputs are host-packed so every dma is 128 FAT descriptors
    (per-partition contiguous), sidestepping the ~100 descr/us HWDGE
    descriptor-generation limit.

    Inputs : xt  [P, KD*C] bf16 — routed tokens (xt[p, kd*C+c] = x[kd*128+p, c])
             w1h [P, KD*256] bf16 — first 256 H-cols of W1, chunk-packed
             w1r [P, 8, KD*512] bf16 — remaining W1 in 8 chunk-packed slabs
                 (last slab half garbage: H-256 = 7.5*512)
             w2r [P, 4, KH*256] bf16 — W2 in 4 chunk-packed slabs
             b1r [P, H/P] f32, b2r [P, D/P] f32 — biases, partition-major
             wc [P, C] f32 — combine weights, replicated across partitions
    Output : yt [D, C] bf16 — w * (gelu(x W1 + b1) W2 + b2), transposed
    """
    key = ("ffn", C)
    if key in _NC_CACHE:
        return _NC_CACHE[key]
    assert C % 4 == 0
    KD = D // P  # 8 k-tiles over D
    KH = H // P  # 32 k-tiles over H
    HEAD = 256
    h_chunks = [HEAD] + [512] * 7 + [256]
    assert sum(h_chunks) == H
    DC = 256  # d columns per W2 dma chunk
    n_off = list(range(0, C, 512))
    n_szs = [min(512, C - o) for o in n_off]
    NCH = len(n_off)

    nc = bacc.Bacc("TRN2", target_bir_lowering=False, debug=False, num_devices=N_CORES)
    xt = nc.dram_tensor("xt", [P, KD * C], BF, kind="ExternalInput")
    w1h = nc.dram_tensor("w1h", [P, KD * HEAD], BF, kind="ExternalInput")
    w1r = nc.dram_tensor("w1r", [P, 8, KD * 512], BF, kind="ExternalInput")
    w2r = nc.dram_tensor("w2r", [P, 4, KH * DC], BF, kind="ExternalInput")
    b1r = nc.dram_tensor("b1r", [P, H // P], F32, kind="ExternalInput")
    b2r = nc.dram_tensor("b2r", [P, D // P], F32, kind="ExternalInput")
    wc = nc.dram_tensor("wc", [P, C], F32, kind="ExternalInput")
    yt = nc.dram_tensor("yt", [D, C], BF, kind="ExternalOutput")

    with tile.TileContext(nc) as tc:
        with (
            tc.tile_pool(name="cst", bufs=1) as cst,
            tc.tile_pool(name="w1p", bufs=3) as w1p,
            tc.tile_pool(name="w2p", bufs=2) as w2p,
            tc.tile_pool(name="outp", bufs=4) as outp,
            tc.tile_pool(name="ps", bufs=4, space="PSUM") as ps,
        ):
            # Startup loads: w1 head first, then thin per-kd xt slices, all
            # on the sync ring — measured faster than splitting across
            # rings (the scalar ring starts later and streams slower).
            w1_c0 = w1p.tile([P, KD * HEAD], BF, tag="w1c0", name="w1_c0")
            xt_sb = cst.tile([P, KD * C], BF)
            nc.sync.dma_start(w1_c0[:], w1h.ap())
            for kd in range(KD):
                nc.sync.dma_start(
                    xt_sb[:, kd * C : (kd + 1) * C],
                    xt.ap()[:, kd * C : (kd + 1) * C],
                )
            b1_sb = cst.tile([P, H // P], F32)
            nc.sync.dma_start(b1_sb[:], b1r.ap())
            b2_sb = cst.tile([P, D // P], F32)
            nc.sync.dma_start(b2_sb[:], b2r.ap())
            wc_sb = cst.tile([P, C], F32)
            nc.gpsimd.dma_start(wc_sb[:], wc.ap())
            ht_sb = cst.tile([P, KH, C], BF)
            # Warm-up: preload the gelu act table (set also covers identity)
            # and ramp the PE p-state with dummy matmuls during the DMA ramp.
            wz = cst.tile([P, 512], BF)
            nc.vector.memset(wz[:], 0.0)
            gld = cst.tile([P, 1], F32)
            nc.scalar.activation(gld[:], wz[:, 0:1], AF.Gelu)
            pw = ps.tile([P, 512], F32, tag="ps1", name="pwarm")
            for i in range(14):
                nc.tensor.matmul(
                    pw[:], wz[:, 0:P], wz[:], start=True, stop=True
                )

            # ---- mm1: ht[h, c] = gelu(sum_d w1[d, h] * xt[d, c] + b1[h]) ----
            h_tile = 0
            for hc, hsz in enumerate(h_chunks):
                if hc == 0:
                    w1_c = w1_c0
                    cs = HEAD  # chunk stride between kd slices
                else:
                    w1_c = w1p.tile([P, KD * 512], BF, tag="w1c", name=f"w1_c{hc}")
                    cs = 512
                    nc.sync.dma_start(w1_c[:], w1r.ap()[:, hc - 1, :])
                for hs in range(hsz // P):
                    psum_ts = [
                        ps.tile([P, 512], F32, tag="ps1", name=f"ps1_{h_tile}_{n}")
                        for n in range(NCH)
                    ]
                    for kd in range(KD):
                        for n in range(NCH):
                            nc.tensor.matmul(
                                psum_ts[n][:, : n_szs[n]],
                                w1_c[:, kd * cs + hs * P : kd * cs + (hs + 1) * P],
                                xt_sb[
                                    :, kd * C + n_off[n] : kd * C + n_off[n] + n_szs[n]
                                ],
                                start=(kd == 0),
                                stop=(kd == KD - 1),
                            )
                    for n in range(NCH):
                        nc.scalar.activation(
                            ht_sb[:, h_tile, n_off[n] : n_off[n] + n_szs[n]],
                            psum_ts[n][:, : n_szs[n]],
                            AF.Gelu,
                            bias=b1_sb[:, h_tile : h_tile + 1],
                        )
                    h_tile += 1

            # ---- mm2: yt[d, c] = (sum_h w2[h, d] * ht[h, c] + b2[d]) * wc[c] ----
            yt_ap = yt.ap().rearrange("(dt p) c -> p dt c", p=P)
            for dc in range(D // DC):
                w2_c = w2p.tile([P, KH * DC], BF, tag="w2c")
                nc.sync.dma_start(w2_c[:], w2r.ap()[:, dc, :])
                for dsx in range(DC // P):
                    d_tile = dc * (DC // P) + dsx
                    psum_ts = [
                        ps.tile([P, 512], F32, tag="ps2", name=f"ps2_{d_tile}_{n}")
                        for n in range(NCH)
                    ]
                    for kh in range(KH):
                        for n in range(NCH):
                            nc.tensor.matmul(
                                psum_ts[n][:, : n_szs[n]],
                                w2_c[:, kh * DC + dsx * P : kh * DC + (dsx + 1) * P],
                                ht_sb[:, kh, n_off[n] : n_off[n] + n_szs[n]],
                                start=(kh == 0),
                                stop=(kh == KH - 1),
                            )
                    out_t = outp.tile([P, C], BF, tag="out")
                    last_dt = d_tile == D // P - 1
                    for n in range(NCH):
                        nsz = n_szs[n]
                        tmp = outp.tile([P, 512], F32, tag="tmp")
                        nc.scalar.activation(
                            tmp[:, :nsz],
                            psum_ts[n][:, :nsz],
                            AF.Identity,
                            bias=b2_sb[:, d_tile : d_tile + 1],
                        )
                        nc.vector.tensor_mul(
                            out_t[:, n_off[n] : n_off[n] + nsz],
                            tmp[:, :nsz],
                            wc_sb[:, n_off[n] : n_off[n] + nsz],
                        )
                        if last_dt:
                            # per-chunk stores so the final transfer overlaps
                            # the tail of the compute chain
                            nc.scalar.dma_start(
                                yt_ap[:, d_tile, n_off[n] : n_off[n] + nsz],
                                out_t[:, n_off[n] : n_off[n] + nsz],
                            )
                    # scalar ring: the sync ring still streams w2 here, and
                    # queueing the outputs behind it delays the final store
                    if not last_dt:
                        nc.scalar.dma_start(yt_ap[:, d_tile, :], out_t[:])
    nc.compile()
    _NC_CACHE[key] = nc
    return nc


# results of the most recent kernel() call, for test harness introspection
last_results = {}


def kernel(**inputs):
    x = np.asarray(inputs["x"], np.float32)
    Wg = np.asarray(inputs["Wg"], np.float32)
    W1 = np.asarray(inputs["W1"], np.float32)
    b1 = np.asarray(inputs["b1"], np.float32)
    W2 = np.asarray(inputs["W2"], np.float32)
    b2 = np.asarray(inputs["b2"], np.float32)
    assert x.shape == (B, S, D) and Wg.shape == (D, E)
    assert W1.shape == (E, D, H) and W2.shape == (E, H, D)

    KD = D // P
    KH = H // P
    TT = TG // P
    HEAD = 256
    xf = np.ascontiguousarray(x.reshape(T, D))
    core_ids = list(range(N_CORES))

    # ---- Launch A: gating on device (data-parallel over tokens) ----
    ncA = _build_gate_nc()
    wgr = np.ascontiguousarray(Wg.reshape(P, KD, E))  # wgr[p,kd,e] = Wg[p*8+kd,e]
    wgh = wgr.astype(BF16)
    wgl = (wgr - wgh.astype(np.float32)).astype(BF16)
    in_maps_a = []
    for m in range(N_CORES):
        xs = np.ascontiguousarray(xf[m * TG : (m + 1) * TG].T)
        xh = xs.astype(BF16)
        xl = (xs - xh.astype(np.float32)).astype(BF16)
        in_maps_a.append({"xhg": xh, "xlg": xl, "wgh": wgh, "wgl": wgl})
    resA = run_bass_kernel_spmd(ncA, in_maps_a, core_ids=core_ids)
    w_full = np.concatenate(
        [
            resA.results[m]["wout"].transpose(1, 0, 2).reshape(TG, E)
            for m in range(N_CORES)
        ],
        axis=0,
    )

    # ---- Host routing: build per-expert token lists from device weights ----
    idx_list, wval_list = [], []
    max_cnt = 1
    for e in range(E):
        idx = np.nonzero(w_full[:, e] > 0.0)[0]
        idx_list.append(idx)
        wval_list.append(w_full[idx, e].astype(np.float32))
        max_cnt = max(max_cnt, len(idx))
    C = ((max_cnt + 3) // 4) * 4

    # ---- Launch B: expert-parallel FFN ----
    ncB = _build_ffn_nc(C)
    in_maps_b = []
    DC = 256
    for e in range(E):
        idx = idx_list[e]
        cnt = len(idx)
        xt = np.zeros((P, KD, C), BF16)
        xt[:, :, :cnt] = (
            xf[idx].T.astype(BF16).reshape(KD, P, cnt).transpose(1, 0, 2)
        )
        wcv = np.zeros((C,), np.float32)
        wcv[:cnt] = wval_list[e]
        w1b = W1[e].astype(BF16)  # [D, H]
        w2b = W2[e].astype(BF16)  # [H, D]
        # w1 rest chunk-packed: [P, 8 slabs, KD*512], last slab half garbage
        w1t = np.concatenate(
            [w1b[:, HEAD:], np.zeros((D, 256), BF16)], axis=1
        )  # [D, 8*512]
        w1r = (
            w1t.reshape(KD, P, 8, 512).transpose(1, 2, 0, 3).reshape(P, 8, KD * 512)
        )
        w2r = (
            w2b.reshape(KH, P, D // DC, DC)
            .transpose(1, 2, 0, 3)
            .reshape(P, D // DC, KH * DC)
        )
        in_maps_b.append(
            {
                "xt": xt.reshape(P, KD * C),
                "w1h": np.ascontiguousarray(
                    w1b[:, :HEAD].reshape(KD, P, HEAD).transpose(1, 0, 2)
                ).reshape(P, KD * HEAD),
                "w1r": np.ascontiguousarray(w1r),
                "w2r": np.ascontiguousarray(w2r),
                "b1r": np.ascontiguousarray(b1[e].reshape(H // P, P).T),
                "b2r": np.ascontiguousarray(b2[e].reshape(D // P, P).T),
                "wc": np.ascontiguousarray(np.broadcast_to(wcv, (P, C))),
            }
        )
    resB = run_bass_kernel_spmd(ncB, in_maps_b, core_ids=core_ids)

    # ---- Host unshard: scatter-add weighted partial outputs ----
    out = np.zeros((T, D), np.float32)
    for e in range(E):
        idx = idx_list[e]
        cnt = len(idx)
        if cnt:
            out[idx] += resB.results[e]["yt"][:, :cnt].T.astype(np.float32)

    last_results["gate"] = resA
    last_results["ffn"] = resB
    return out.reshape(B, S, D)
